# revision 1
# baseline (speedup 1.0000x reference)
"""ChebConv (K=3) Trainium2 kernel, 8-core SPMD — SBUF-source gather design.

Math: with lam = lambda_max, c1=-2/lam, c2=2/lam-1, d1=-4/lam, d2=4/lam-2 and
A = D^-1/2 A D^-1/2 (in-degree norm, clamped), the reference output is

    out = feat @ M0 + g @ M1 + q @ M2 + bias,   g = A feat, q = A g
    M0 = W0^T + c2 W1^T + (d2 c2 - 1) W2^T
    M1 = c1 W1^T + (d1 c2 + d2 c1) W2^T
    M2 = d1 c1 W2^T

Device strategy (one NEFF, SPMD on 8 cores):
  - dst nodes padded to a multiple of 8*128 and block-partitioned; 98 dst
    blocks per core. Edges bucketed by (dst block, src chunk) on host; each
    bucket padded to a multiple of 128 "edge tiles" (max over cores so the
    program is shared).
  - per hop, chunk-outer streaming: each src chunk (25088 rows fp16) is
    DMA'd CONTIGUOUSLY into SBUF (large descriptors, full HBM BW), then
    edge-source rows are gathered SBUF->SBUF with gpsimd.dma_gather
    (sbuf_tokens_per_rank=128), avoiding the HBM random-256B-read latency
    wall. A host-side index remap idx(r) = (r//RPP) + 128*(r%RPP) makes the
    contiguous chunk layout match the gather's token addressing.
  - SBUF-source gather emits feature-major tiles [128f x e]; each 128-edge
    tile is transposed back to edge-major via an identity matmul (batched 4
    tiles per PSUM bank + one fused PSUM->SBUF copy).
  - per edge tile: one fused tensor_scalar builds the weighted one-hot
    (iota == dl) * w, w = norm[src]*norm[dst] (0 for padding); matmul
    lhsT=X_tile rhs=onehot accumulates the (dst block, chunk) partial in
    PSUM, which drains into a resident SBUF accumulator (gT for hop 1,
    qS for hop 2) via copy/add.
  - hop-1 accumulators are transposed back to node-major via an identity
    matmul and written to a DRAM bounce buffer; one fp16 AllGather shares
    g across cores; hop 2 repeats the structure reading the AllGather
    output. Dense epilogue per block on TensorE with host-folded M0/M1/M2.
"""
import os
import sys

sys.path.insert(0, "/opt/trn_rl_repo")

import numpy as np

import concourse.bacc as bacc
import concourse.mybir as mybir
import concourse.tile as tile
from concourse import bass_utils

NCORE = 8
BLK = 128
D = 128
NCHUNK = 8
CALL_TILES = 32                      # edge tiles per dma_gather call
CALL_IDX = CALL_TILES * BLK
TBATCH = 4                           # tiles per PSUM transpose-stage batch


def _prep(feat, W, bias, lambda_max, src, dst):
    """Host-side graph preprocessing. Returns per-core in_maps + plan."""
    N = feat.shape[0]
    E = src.shape[0]
    src = np.asarray(src).astype(np.int64)
    dst = np.asarray(dst).astype(np.int64)
    feat = np.asarray(feat).astype(np.float32)
    W = np.asarray(W).astype(np.float32)
    bias = np.asarray(bias).astype(np.float32)
    lam = float(np.asarray(lambda_max).reshape(-1)[0])

    npad_unit = NCORE * BLK
    NPAD = ((N + npad_unit - 1) // npad_unit) * npad_unit
    NBLK = NPAD // BLK
    BPC = NBLK // NCORE
    NPC = BPC * BLK
    CHUNK = NPAD // NCHUNK
    RPP = CHUNK // 128                # rows per partition in a chunk tile
    assert CHUNK % 128 == 0 and CHUNK - 1 < 32767, (NPAD, CHUNK)

    # normalization
    deg = np.bincount(dst, minlength=N).astype(np.float32)
    norm = np.clip(deg, 1.0, None) ** -0.5
    w_all = (norm[src] * norm[dst]).astype(np.float32)

    blk_all = dst // BLK                      # global dst block
    chunk_all = src // CHUNK
    key = (blk_all * NCHUNK + chunk_all).astype(np.int64)
    order = np.argsort(key, kind="stable")
    sk = key[order]

    cnt_flat = np.bincount(key, minlength=NBLK * NCHUNK)
    cnt = cnt_flat.reshape(NCORE, BPC, NCHUNK)
    # tiles per (block-within-core, chunk): max over cores -> shared program
    T = -(-cnt.max(axis=0) // BLK)            # [BPC, NCHUNK]
    # every block needs at least one tile so its accumulator gets written
    none_mask = T.sum(axis=1) == 0
    T[none_mask, 0] = 1
    tile_off = np.zeros((BPC, NCHUNK), np.int64)
    NT = np.zeros(NCHUNK, np.int64)
    for c in range(NCHUNK):
        tile_off[:, c] = np.cumsum(T[:, c]) - T[:, c]
        NT[c] = T[:, c].sum()

    # slot position of every edge inside its core's per-chunk stream
    group_starts = np.zeros(NBLK * NCHUNK + 1, np.int64)
    group_starts[1:] = np.cumsum(cnt_flat)
    rank = np.arange(E, dtype=np.int64) - group_starts[sk]
    bb_s = (sk // NCHUNK) % BPC
    core_s = (sk // NCHUNK) // BPC
    c_s = sk % NCHUNK
    pos = tile_off[bb_s, c_s] * BLK + rank

    # chunk-local row -> SBUF-gather token id under a CONTIGUOUS chunk load:
    # row r lives at partition r//RPP, byte offset (r%RPP)*256; the gather
    # decodes idx as (partition = idx%128, rank-slot = idx//128).
    rloc = (src - chunk_all * CHUNK).astype(np.int64)
    idx16_all = ((rloc // RPP) + 128 * (rloc % RPP)).astype(np.int16)[order]
    w_s = w_all[order]
    dl_s = (dst % BLK).astype(np.float32)[order]

    idxs = [np.zeros((NCORE, NT[c] * BLK), np.int16) for c in range(NCHUNK)]
    ws = [np.zeros((NCORE, NT[c] * BLK), np.float32) for c in range(NCHUNK)]
    dls = [np.zeros((NCORE, NT[c] * BLK), np.float32) for c in range(NCHUNK)]
    for c in range(NCHUNK):
        m = c_s == c
        idxs[c][core_s[m], pos[m]] = idx16_all[m]
        ws[c][core_s[m], pos[m]] = w_s[m]
        dls[c][core_s[m], pos[m]] = dl_s[m]

    # folded dense matrices
    c1 = -2.0 / lam
    c2 = 2.0 / lam - 1.0
    d1 = -4.0 / lam
    d2 = 4.0 / lam - 2.0
    W0T, W1T, W2T = W[0].T, W[1].T, W[2].T
    M0 = W0T + c2 * W1T + (d2 * c2 - 1.0) * W2T
    M1 = c1 * W1T + (d1 * c2 + d2 * c1) * W2T
    M2 = (d1 * c1) * W2T

    featH = np.zeros((NPAD, D), np.float16)
    featH[:N] = feat.astype(np.float16)

    # first/last nonempty chunk per block (shared across cores)
    first_c = np.zeros(BPC, np.int64)
    last_c = np.zeros(BPC, np.int64)
    for bb in range(BPC):
        nz = np.nonzero(T[bb])[0]
        first_c[bb] = nz[0]
        last_c[bb] = nz[-1]

    shared = {
        "M0": M0.astype(np.float16),
        "M1": M1.astype(np.float16),
        "M2": M2.astype(np.float16),
        "bias_rep": np.tile(bias[None, :].astype(np.float32), (BLK, 1)),
        "iota": np.tile(np.arange(BLK, dtype=np.float16)[None, :], (BLK, 1)),
        "ident": np.eye(BLK, dtype=np.float16),
        "featH": featH.reshape(NCHUNK * 128, CHUNK),
    }
    in_maps = []
    for k in range(NCORE):
        m = dict(shared)
        m["featLocal"] = featH[k * NPC : (k + 1) * NPC]
        for c in range(NCHUNK):
            m[f"idx{c}"] = np.ascontiguousarray(
                np.tile(idxs[c][k].reshape(-1, 16).T, (8, 1))
            )
            m[f"w{c}"] = np.ascontiguousarray(ws[c][k].reshape(-1, BLK).T)
            m[f"dl{c}"] = np.ascontiguousarray(dls[c][k].reshape(-1, BLK).T)
        in_maps.append(m)

    plan = dict(N=N, NPAD=NPAD, BPC=BPC, NPC=NPC, CHUNK=CHUNK, RPP=RPP,
                T=T, tile_off=tile_off, NT=NT, first_c=first_c, last_c=last_c)
    return in_maps, plan


def _build(plan, variant="full", reps=1):
    """Emit the Bass/Tile program for the shared SPMD NEFF.

    variant="full": the real kernel (hop1 -> AllGather -> hop2+epilogue).
    variant="timing_*": no collective; hops wrapped in a For_i(reps)
    hardware loop for differential wall-clock timing.
    """
    BPC, NPC, NPAD, CHUNK = plan["BPC"], plan["NPC"], plan["NPAD"], plan["CHUNK"]
    T, tile_off, NT = plan["T"], plan["tile_off"], plan["NT"]
    first_c, last_c = plan["first_c"], plan["last_c"]
    f16, f32, i16 = mybir.dt.float16, mybir.dt.float32, mybir.dt.int16

    nc = bacc.Bacc("TRN2", target_bir_lowering=False, debug=False,
                   num_devices=NCORE, num_swdge_queues=4)
    featH_d = nc.dram_tensor("featH", [NCHUNK * 128, CHUNK], f16,
                             kind="ExternalInput")
    featL_d = nc.dram_tensor("featLocal", [NPC, D], f16, kind="ExternalInput")
    idx_d = [nc.dram_tensor(f"idx{c}", [128, NT[c] * 8], i16, kind="ExternalInput")
             for c in range(NCHUNK)]
    w_d = [nc.dram_tensor(f"w{c}", [128, NT[c]], f32, kind="ExternalInput")
           for c in range(NCHUNK)]
    dl_d = [nc.dram_tensor(f"dl{c}", [128, NT[c]], f32, kind="ExternalInput")
            for c in range(NCHUNK)]
    M_d = [nc.dram_tensor(f"M{i}", [D, D], f16, kind="ExternalInput")
           for i in range(3)]
    bias_d = nc.dram_tensor("bias_rep", [BLK, D], f32, kind="ExternalInput")
    iota_d = nc.dram_tensor("iota", [BLK, BLK], f16, kind="ExternalInput")
    ident_d = nc.dram_tensor("ident", [BLK, BLK], f16, kind="ExternalInput")
    out_d = nc.dram_tensor("out", [NPC, D], f32, kind="ExternalOutput")

    skip_gather = variant == "timing_compute"
    skip_compute = variant == "timing_gather"

    with tile.TileContext(nc) as tc:
        with (
            tc.tile_pool(name="const", bufs=1) as cpool,
            tc.tile_pool(name="resident", bufs=1) as rpool,
            tc.tile_pool(name="chunkp", bufs=1) as chpool,
            tc.tile_pool(name="idxp", bufs=4) as idxpool,
            tc.tile_pool(name="streams", bufs=2) as spool,
            tc.tile_pool(name="lhsp", bufs=3) as lpool,
            tc.tile_pool(name="ow", bufs=8) as owpool,
            tc.tile_pool(name="small", bufs=3) as npool,
            tc.tile_pool(name="psum", bufs=1, space="PSUM") as psum,
            tc.tile_pool(name="psum_stage", bufs=2, space="PSUM") as stpsum,
            tc.tile_pool(name="psum_acc", bufs=3, space="PSUM") as acpsum,
            tc.tile_pool(name="dram", bufs=1, space="DRAM") as dram,
        ):
            iota_t = cpool.tile([BLK, BLK], f16)
            nc.sync.dma_start(out=iota_t[:], in_=iota_d[:])
            ident_t = cpool.tile([BLK, BLK], f16)
            nc.sync.dma_start(out=ident_t[:], in_=ident_d[:])
            M_t = []
            for i in range(3):
                t = cpool.tile([D, D], f16, tag=f"M{i}")
                nc.sync.dma_start(out=t[:], in_=M_d[i][:])
                M_t.append(t)
            bias_t = cpool.tile([BLK, D], f32)
            nc.sync.dma_start(out=bias_t[:], in_=bias_d[:])
            w_t, dl_t = [], []
            for c in range(NCHUNK):
                wt = rpool.tile([128, NT[c]], f32, tag=f"w{c}")
                nc.sync.dma_start(out=wt[:], in_=w_d[c][:])
                w_t.append(wt)
                dt_ = rpool.tile([128, NT[c]], f32, tag=f"dl{c}")
                nc.sync.dma_start(out=dt_[:], in_=dl_d[c][:])
                dl_t.append(dt_)
            featT = rpool.tile([128, NPC], f16, tag="featT")
            nc.sync.dma_start_transpose(out=featT[:], in_=featL_d[:])
            gT = rpool.tile([128, NPC], f16, tag="gT")
            qS = rpool.tile([128, NPC], f16, tag="qS")

            cc_in = dram.tile([NPC, D], f16)
            cc_out = dram.tile([NCHUNK * 128, CHUNK], f16)

            # position -> owning dst block, within each chunk's tile stream
            p2bb = []
            for c in range(NCHUNK):
                arr = np.zeros(int(NT[c]), np.int64)
                for bb in range(BPC):
                    o = int(tile_off[bb][c])
                    arr[o : o + int(T[bb][c])] = bb
                p2bb.append(arr)

            def run_hop(src_chunk_views, accT, out_hook):
                """One SpMM hop: accT[:, bb*128:(bb+1)*128] = sum over chunks
                of the (bb, c) PSUM partials; out_hook(bb) emitted after the
                last chunk of bb drains."""
                for c in range(NCHUNK):
                    ntc = int(NT[c])
                    ncalls = -(-ntc // CALL_TILES)
                    # contiguous chunk load (token-major layout by construction)
                    if not skip_gather:
                        ch = chpool.tile([128, CHUNK], f16, tag="chunk")
                        nc.sync.dma_start(out=ch[:], in_=src_chunk_views[c])
                    gbufs = {}

                    def ensure_call(j):
                        if j in gbufs or skip_gather:
                            return
                        n_t = min(CALL_TILES, ntc - j * CALL_TILES)
                        n_idx = n_t * BLK
                        ib = idxpool.tile([128, CALL_IDX // 16], i16, tag="idx")
                        nc.sync.dma_start(
                            out=ib[:, : n_idx // 16],
                            in_=idx_d[c][:, j * (CALL_IDX // 16):
                                         j * (CALL_IDX // 16) + n_idx // 16],
                        )
                        gb = spool.tile([128, 1, CALL_IDX], f16, tag="g")
                        nc.gpsimd.dma_gather(
                            out_ap=gb[:, :, :n_idx],
                            in_ap=ch[:],
                            idxs_ap=ib[:, : n_idx // 16],
                            num_idxs=n_idx,
                            num_idxs_reg=n_idx,
                            elem_size=D,
                            transpose=True,
                            single_packet=False,
                            sbuf_tokens_per_rank=128,
                            sbuf_free_dim_per_rank=D * 2,
                            queue_num=c % 4,
                        )
                        gbufs[j] = gb

                    if skip_compute:
                        for j in range(ncalls):
                            ensure_call(j)
                        continue

                    # static fake gather bufs for timing_compute
                    if skip_gather:
                        for b in range(2):
                            gb = spool.tile([128, 1, CALL_IDX], f16, tag="g")
                            nc.vector.memset(out=gb[:], value=0.0)
                            gbufs[b] = gb

                    lhs_sb = None
                    for p in range(ntc):
                        j, slot = divmod(p, CALL_TILES)
                        if skip_gather:
                            gb = gbufs[j % 2]
                        else:
                            ensure_call(j)
                            gb = gbufs[j]
                        # transpose-stage batch of TBATCH tiles
                        k = p % TBATCH
                        if k == 0:
                            nb = min(TBATCH, ntc - p)
                            stage = stpsum.tile([128, TBATCH * BLK], f32,
                                                tag="stage", space="PSUM")
                            for kk in range(nb):
                                pp = p + kk
                                jj, ss = divmod(pp, CALL_TILES)
                                if skip_gather:
                                    gb2 = gbufs[jj % 2]
                                else:
                                    ensure_call(jj)
                                    gb2 = gbufs[jj]
                                nc.tensor.matmul(
                                    out=stage[:, kk * BLK : (kk + 1) * BLK],
                                    lhsT=gb2[:, 0, ss * BLK : (ss + 1) * BLK],
                                    rhs=ident_t[:],
                                    start=True, stop=True,
                                )
                            lhs_sb = lpool.tile([128, TBATCH * BLK], f16, tag="lhs")
                            nc.vector.tensor_copy(out=lhs_sb[:, : nb * BLK],
                                                  in_=stage[:, : nb * BLK])
                        bb = int(p2bb[c][p])
                        start = p == int(tile_off[bb][c])
                        stop = p == int(tile_off[bb][c]) + int(T[bb][c]) - 1
                        if start:
                            acc = acpsum.tile([128, BLK], f32, tag="acc",
                                              space="PSUM")
                        ow = owpool.tile([128, BLK], f16, tag="ow")
                        nc.vector.tensor_scalar(
                            out=ow[:],
                            in0=iota_t[:],
                            scalar1=dl_t[c][:, p : p + 1],
                            scalar2=w_t[c][:, p : p + 1],
                            op0=mybir.AluOpType.is_equal,
                            op1=mybir.AluOpType.mult,
                        )
                        nc.tensor.matmul(
                            out=acc[:],
                            lhsT=lhs_sb[:, k * BLK : (k + 1) * BLK],
                            rhs=ow[:],
                            start=start,
                            stop=stop,
                        )
                        if stop:
                            sl = slice(bb * BLK, (bb + 1) * BLK)
                            if c == int(first_c[bb]):
                                nc.vector.tensor_copy(out=accT[:, sl], in_=acc[:])
                            else:
                                nc.vector.tensor_tensor(
                                    out=accT[:, sl], in0=accT[:, sl], in1=acc[:],
                                    op=mybir.AluOpType.add)
                            if c == int(last_c[bb]):
                                out_hook(bb)

            # ---- hop 1: g = A feat ----
            def hop1_out(bb):
                sl = slice(bb * BLK, (bb + 1) * BLK)
                tp = psum.tile([128, BLK], f32, tag="tp", space="PSUM")
                nc.tensor.matmul(out=tp[:], lhsT=gT[:, sl], rhs=ident_t[:],
                                 start=True, stop=True)
                gn = npool.tile([BLK, D], f16, tag="gn")
                nc.vector.tensor_copy(out=gn[:], in_=tp[:])
                nc.sync.dma_start(out=cc_in[sl, :], in_=gn[:])

            # ---- hop 2: q = A g, fused epilogue ----
            def hop2_out(bb):
                sl = slice(bb * BLK, (bb + 1) * BLK)
                out_ps = psum.tile([128, BLK], f32, tag="outp", space="PSUM")
                nc.tensor.matmul(out=out_ps[:], lhsT=featT[:, sl], rhs=M_t[0][:],
                                 start=True, stop=False)
                nc.tensor.matmul(out=out_ps[:], lhsT=gT[:, sl], rhs=M_t[1][:],
                                 start=False, stop=False)
                nc.tensor.matmul(out=out_ps[:], lhsT=qS[:, sl], rhs=M_t[2][:],
                                 start=False, stop=True)
                ob = npool.tile([BLK, D], f32, tag="ob")
                nc.vector.tensor_tensor(out=ob[:], in0=out_ps[:], in1=bias_t[:],
                                        op=mybir.AluOpType.add)
                nc.sync.dma_start(out=out_d[sl, :], in_=ob[:])

            def hops_body():
                run_hop(
                    [featH_d[c * 128 : (c + 1) * 128, :] for c in range(NCHUNK)],
                    gT,
                    hop1_out,
                )
                if variant == "full":
                    nc.gpsimd.collective_compute(
                        "AllGather",
                        mybir.AluOpType.bypass,
                        ins=[cc_in.opt()],
                        outs=[cc_out.opt()],
                        replica_groups=[list(range(NCORE))],
                    )
                h2_src = featH_d if variant == "debug_nocc" else cc_out
                run_hop(
                    [h2_src[c * 128 : (c + 1) * 128, :] for c in range(NCHUNK)],
                    qS,
                    hop2_out,
                )

            if variant.startswith("timing") and reps > 1:
                with tc.For_i(0, reps, 1):
                    hops_body()
            else:
                hops_body()

    nc.compile()
    return nc


def kernel(feat, W, bias, lambda_max, src, dst):
    in_maps, plan = _prep(feat, W, bias, lambda_max, src, dst)
    nc = _build(plan)
    res = bass_utils.run_bass_kernel_spmd(nc, in_maps, core_ids=list(range(NCORE)))
    # stashed for external benchmarking harnesses (not used by the kernel)
    kernel.last_nc = nc
    kernel.last_in_maps = in_maps
    kernel.last_plan = plan
    out = np.concatenate([res.results[k]["out"] for k in range(NCORE)], axis=0)
    return np.ascontiguousarray(out[: plan["N"]]).astype(np.float32)



# revision 7
# speedup vs baseline: 1.6382x; 1.6382x over previous
"""ChebConv (K=3) Trainium2 kernel, 8-core SPMD — HBM-gather design.

Math: with lam = lambda_max, c1=-2/lam, c2=2/lam-1, d1=-4/lam, d2=4/lam-2 and
A = D^-1/2 A D^-1/2 (in-degree norm, clamped), the reference output is

    out = feat @ M0 + g @ M1 + q @ M2 + bias,   g = A feat, q = A g
    M0 = W0^T + c2 W1^T + (d2 c2 - 1) W2^T
    M1 = c1 W1^T + (d1 c2 + d2 c1) W2^T
    M2 = d1 c1 W2^T

Device strategy (one NEFF, SPMD on 8 cores):
  - dst nodes padded to a multiple of 8*128 and block-partitioned; 98 dst
    blocks per core. Edges bucketed by (dst block, src quarter) on host; each
    bucket padded to a multiple of 128 "edge tiles" (max over cores so the
    program is shared). NQUAR=4 quarters of 25088 src nodes respect the
    int16 gather index limit; bucket padding ~13%.
  - per hop, edge-source rows are gathered DIRECTLY FROM HBM (node-major
    feature table) with gpsimd.dma_gather(transpose=False), which emits
    EDGE-MAJOR tiles [128e x 128f] consumed by the scatter matmul with no
    further data movement. KEY perf facts (measured):
      * gather descriptor generation is Q7-core bound; SWDGE queue q is
        served by Q7 core pair (2q, 2q+1), so calls ROTATE across all 4
        queues => ~4x descriptor throughput (2.3 ns/idx vs 9 ns/idx).
      * transpose-mode gathers CORRUPT when run concurrently on different
        queues (shared XBAR spray state); non-transpose gathers are safe.
  - per edge tile: one fused DVE tensor_scalar builds the weighted one-hot
    (iota == dl) * w, w = norm[src]*norm[dst] (0 for padding); matmul
    lhsT=X_tile[e,f] rhs=onehot[e,d] accumulates the (dst block, quarter)
    partial in PSUM [f,d], which drains into a resident SBUF accumulator
    (gT for hop 1, qS for hop 2): first quarter via ACT copy, later
    quarters via DVE add.
  - hop-1 accumulators are transposed back to node-major via an identity
    matmul and written to a DRAM bounce buffer; one fp16 AllGather shares
    g across cores; hop 2 repeats the structure gathering from the
    AllGather output. Dense epilogue per block on TensorE with host-folded
    M0/M1/M2.
"""
import os
import sys

sys.path.insert(0, "/opt/trn_rl_repo")

import numpy as np

import concourse.bacc as bacc
import concourse.mybir as mybir
import concourse.tile as tile
from concourse import bass_utils

NCORE = 8
BLK = 128
D = 128
NQUAR = 4
NQUEUE = 4
CALL_TILES = 32                      # edge tiles per dma_gather call
CALL_IDX = CALL_TILES * BLK


def _prep(feat, W, bias, lambda_max, src, dst):
    """Host-side graph preprocessing. Returns per-core in_maps + plan."""
    N = feat.shape[0]
    E = src.shape[0]
    src = np.asarray(src).astype(np.int64)
    dst = np.asarray(dst).astype(np.int64)
    feat = np.asarray(feat).astype(np.float32)
    W = np.asarray(W).astype(np.float32)
    bias = np.asarray(bias).astype(np.float32)
    lam = float(np.asarray(lambda_max).reshape(-1)[0])

    npad_unit = NCORE * BLK
    NPAD = ((N + npad_unit - 1) // npad_unit) * npad_unit
    NBLK = NPAD // BLK
    BPC = NBLK // NCORE
    NPC = BPC * BLK
    QR = NPAD // NQUAR                # rows per quarter table
    assert QR - 1 < 32767, (NPAD, QR)

    # normalization
    deg = np.bincount(dst, minlength=N).astype(np.float32)
    norm = np.clip(deg, 1.0, None) ** -0.5
    w_all = (norm[src] * norm[dst]).astype(np.float32)

    blk_all = dst // BLK                      # global dst block
    quar_all = src // QR
    key = (blk_all * NQUAR + quar_all).astype(np.int64)
    order = np.argsort(key, kind="stable")
    sk = key[order]

    cnt_flat = np.bincount(key, minlength=NBLK * NQUAR)
    cnt = cnt_flat.reshape(NCORE, BPC, NQUAR)
    # tiles per (block-within-core, quarter): max over cores -> shared program
    T = -(-cnt.max(axis=0) // BLK)            # [BPC, NQUAR]
    # every block needs at least one tile so its accumulator gets written
    none_mask = T.sum(axis=1) == 0
    T[none_mask, 0] = 1
    tile_off = np.zeros((BPC, NQUAR), np.int64)
    NT = np.zeros(NQUAR, np.int64)
    for c in range(NQUAR):
        tile_off[:, c] = np.cumsum(T[:, c]) - T[:, c]
        NT[c] = T[:, c].sum()

    # slot position of every edge inside its core's per-quarter stream
    group_starts = np.zeros(NBLK * NQUAR + 1, np.int64)
    group_starts[1:] = np.cumsum(cnt_flat)
    rank = np.arange(E, dtype=np.int64) - group_starts[sk]
    bb_s = (sk // NQUAR) % BPC
    core_s = (sk // NQUAR) // BPC
    c_s = sk % NQUAR
    pos = tile_off[bb_s, c_s] * BLK + rank

    # quarter-local row index (the HBM gather idx)
    idx16_all = (src - quar_all * QR).astype(np.int16)[order]
    w_s = w_all[order]
    dl_s = (dst % BLK).astype(np.float32)[order]

    idxs = [np.zeros((NCORE, NT[c] * BLK), np.int16) for c in range(NQUAR)]
    ws = [np.zeros((NCORE, NT[c] * BLK), np.float32) for c in range(NQUAR)]
    dls = [np.zeros((NCORE, NT[c] * BLK), np.float32) for c in range(NQUAR)]
    for c in range(NQUAR):
        m = c_s == c
        idxs[c][core_s[m], pos[m]] = idx16_all[m]
        ws[c][core_s[m], pos[m]] = w_s[m]
        dls[c][core_s[m], pos[m]] = dl_s[m]

    # folded dense matrices
    c1 = -2.0 / lam
    c2 = 2.0 / lam - 1.0
    d1 = -4.0 / lam
    d2 = 4.0 / lam - 2.0
    W0T, W1T, W2T = W[0].T, W[1].T, W[2].T
    M0 = W0T + c2 * W1T + (d2 * c2 - 1.0) * W2T
    M1 = c1 * W1T + (d1 * c2 + d2 * c1) * W2T
    M2 = (d1 * c1) * W2T

    featH = np.zeros((NPAD, D), np.float16)
    featH[:N] = feat.astype(np.float16)

    # first/last nonempty quarter per block (shared across cores)
    first_c = np.zeros(BPC, np.int64)
    last_c = np.zeros(BPC, np.int64)
    for bb in range(BPC):
        nz = np.nonzero(T[bb])[0]
        first_c[bb] = nz[0]
        last_c[bb] = nz[-1]

    shared = {
        "M0": M0.astype(np.float16),
        "M1": M1.astype(np.float16),
        "M2": M2.astype(np.float16),
        "bias_rep": np.tile(bias[None, :].astype(np.float32), (BLK, 1)),
        "iota": np.tile(np.arange(BLK, dtype=np.float16)[None, :], (BLK, 1)),
        "ident": np.eye(BLK, dtype=np.float16),
        "featH": featH,
    }
    in_maps = []
    for k in range(NCORE):
        m = dict(shared)
        m["featLocal"] = featH[k * NPC : (k + 1) * NPC]
        for c in range(NQUAR):
            m[f"idx{c}"] = np.ascontiguousarray(
                np.tile(idxs[c][k].reshape(-1, 16).T, (8, 1))
            )
            m[f"w{c}"] = np.ascontiguousarray(ws[c][k].reshape(-1, BLK).T)
            m[f"dl{c}"] = np.ascontiguousarray(dls[c][k].reshape(-1, BLK).T)
        in_maps.append(m)

    plan = dict(N=N, NPAD=NPAD, BPC=BPC, NPC=NPC, QR=QR,
                T=T, tile_off=tile_off, NT=NT, first_c=first_c, last_c=last_c)
    return in_maps, plan


def _build(plan, variant="full", reps=1):
    """Emit the Bass/Tile program for the shared SPMD NEFF.

    variant="full": the real kernel (hop1 -> AllGather -> hop2+epilogue).
    variant="timing_*": no collective; hops wrapped in a For_i(reps)
    hardware loop for differential wall-clock timing.
    """
    BPC, NPC, NPAD, QR = plan["BPC"], plan["NPC"], plan["NPAD"], plan["QR"]
    T, tile_off, NT = plan["T"], plan["tile_off"], plan["NT"]
    first_c, last_c = plan["first_c"], plan["last_c"]
    f16, f32, i16 = mybir.dt.float16, mybir.dt.float32, mybir.dt.int16

    nc = bacc.Bacc("TRN2", target_bir_lowering=False, debug=False,
                   num_devices=NCORE, num_swdge_queues=NQUEUE)
    featH_d = nc.dram_tensor("featH", [NPAD, D], f16, kind="ExternalInput")
    featL_d = nc.dram_tensor("featLocal", [NPC, D], f16, kind="ExternalInput")
    idx_d = [nc.dram_tensor(f"idx{c}", [128, NT[c] * 8], i16, kind="ExternalInput")
             for c in range(NQUAR)]
    w_d = [nc.dram_tensor(f"w{c}", [128, NT[c]], f32, kind="ExternalInput")
           for c in range(NQUAR)]
    dl_d = [nc.dram_tensor(f"dl{c}", [128, NT[c]], f32, kind="ExternalInput")
            for c in range(NQUAR)]
    M_d = [nc.dram_tensor(f"M{i}", [D, D], f16, kind="ExternalInput")
           for i in range(3)]
    bias_d = nc.dram_tensor("bias_rep", [BLK, D], f32, kind="ExternalInput")
    iota_d = nc.dram_tensor("iota", [BLK, BLK], f16, kind="ExternalInput")
    ident_d = nc.dram_tensor("ident", [BLK, BLK], f16, kind="ExternalInput")
    out_d = nc.dram_tensor("out", [NPC, D], f32, kind="ExternalOutput")

    skip_gather = variant == "timing_compute"
    skip_compute = variant == "timing_gather"

    with tile.TileContext(nc) as tc:
        with (
            tc.tile_pool(name="const", bufs=1) as cpool,
            tc.tile_pool(name="resident", bufs=1) as rpool,
            tc.tile_pool(name="idxp", bufs=8) as idxpool,
            tc.tile_pool(name="streams", bufs=8) as spool,
            tc.tile_pool(name="ow", bufs=8) as owpool,
            tc.tile_pool(name="small", bufs=3) as npool,
            tc.tile_pool(name="psum", bufs=1, space="PSUM") as psum,
            tc.tile_pool(name="psum_acc", bufs=4, space="PSUM") as acpsum,
            tc.tile_pool(name="dram", bufs=1, space="DRAM") as dram,
        ):
            iota_t = cpool.tile([BLK, BLK], f16)
            nc.sync.dma_start(out=iota_t[:], in_=iota_d[:])
            ident_t = cpool.tile([BLK, BLK], f16)
            nc.sync.dma_start(out=ident_t[:], in_=ident_d[:])
            M_t = []
            for i in range(3):
                t = cpool.tile([D, D], f16, tag=f"M{i}")
                nc.sync.dma_start(out=t[:], in_=M_d[i][:])
                M_t.append(t)
            bias_t = cpool.tile([BLK, D], f32)
            nc.sync.dma_start(out=bias_t[:], in_=bias_d[:])
            w_t, dl_t = [], []
            for c in range(NQUAR):
                wt = rpool.tile([128, NT[c]], f32, tag=f"w{c}")
                nc.sync.dma_start(out=wt[:], in_=w_d[c][:])
                w_t.append(wt)
                dt_ = rpool.tile([128, NT[c]], f32, tag=f"dl{c}")
                nc.sync.dma_start(out=dt_[:], in_=dl_d[c][:])
                dl_t.append(dt_)
            featT = rpool.tile([128, NPC], f16, tag="featT")
            nc.sync.dma_start_transpose(out=featT[:], in_=featL_d[:])
            gT = rpool.tile([128, NPC], f16, tag="gT")
            qS = rpool.tile([128, NPC], f16, tag="qS")

            cc_in = dram.tile([NPC, D], f16)
            cc_out = dram.tile([NPAD, D], f16)

            # position -> owning dst block, within each quarter's tile stream
            p2bb = []
            for c in range(NQUAR):
                arr = np.zeros(int(NT[c]), np.int64)
                for bb in range(BPC):
                    o = int(tile_off[bb][c])
                    arr[o : o + int(T[bb][c])] = bb
                p2bb.append(arr)

            def run_hop(src_table, accT, out_hook):
                """One SpMM hop: accT[:, bb*128:(bb+1)*128] = sum over
                quarters of the (bb, c) PSUM partials; out_hook(bb) emitted
                after the last quarter of bb drains."""
                qcounter = [0]
                for c in range(NQUAR):
                    ntc = int(NT[c])
                    ncalls = -(-ntc // CALL_TILES)
                    table_view = src_table[c * QR : (c + 1) * QR, :]
                    gbufs = {}

                    def ensure_call(j):
                        if j in gbufs or skip_gather:
                            return
                        n_t = min(CALL_TILES, ntc - j * CALL_TILES)
                        n_idx = n_t * BLK
                        ib = idxpool.tile([128, CALL_IDX // 16], i16, tag="idx")
                        nc.sync.dma_start(
                            out=ib[:, : n_idx // 16],
                            in_=idx_d[c][:, j * (CALL_IDX // 16):
                                         j * (CALL_IDX // 16) + n_idx // 16],
                        )
                        gb = spool.tile([128, CALL_TILES, D], f16, tag="g")
                        nc.gpsimd.dma_gather(
                            out_ap=gb[:, :n_t, :],
                            in_ap=table_view,
                            idxs_ap=ib[:, : n_idx // 16],
                            num_idxs=n_idx,
                            num_idxs_reg=n_idx,
                            elem_size=D,
                            transpose=False,
                            single_packet=False,
                            queue_num=qcounter[0] % NQUEUE,
                        )
                        qcounter[0] += 1
                        gbufs[j] = gb

                    if skip_compute:
                        for j in range(ncalls):
                            ensure_call(j)
                        continue

                    # static fake gather bufs for timing_compute
                    if skip_gather:
                        for b in range(2):
                            gb = spool.tile([128, CALL_TILES, D], f16, tag="g")
                            nc.vector.memset(gb[:], 0.0)
                            gbufs[b] = gb

                    for p in range(ntc):
                        j, slot = divmod(p, CALL_TILES)
                        if skip_gather:
                            gb = gbufs[j % 2]
                        else:
                            ensure_call(j)
                            gb = gbufs[j]
                        bb = int(p2bb[c][p])
                        start = p == int(tile_off[bb][c])
                        stop = p == int(tile_off[bb][c]) + int(T[bb][c]) - 1
                        if start:
                            acc = acpsum.tile([128, BLK], f32, tag="acc",
                                              space="PSUM")
                        ow = owpool.tile([128, BLK], f16, tag="ow")
                        nc.vector.tensor_scalar(
                            out=ow[:],
                            in0=iota_t[:],
                            scalar1=dl_t[c][:, p : p + 1],
                            scalar2=w_t[c][:, p : p + 1],
                            op0=mybir.AluOpType.is_equal,
                            op1=mybir.AluOpType.mult,
                        )
                        nc.tensor.matmul(
                            out=acc[:],
                            lhsT=gb[:, slot, :],
                            rhs=ow[:],
                            start=start,
                            stop=stop,
                        )
                        if stop:
                            sl = slice(bb * BLK, (bb + 1) * BLK)
                            if c == int(first_c[bb]):
                                nc.scalar.copy(out=accT[:, sl], in_=acc[:])
                            else:
                                nc.vector.tensor_tensor(
                                    out=accT[:, sl], in0=accT[:, sl], in1=acc[:],
                                    op=mybir.AluOpType.add)
                            if c == int(last_c[bb]):
                                out_hook(bb)

            # ---- hop 1: g = A feat ----
            def hop1_out(bb):
                sl = slice(bb * BLK, (bb + 1) * BLK)
                tp = psum.tile([128, BLK], f32, tag="tp", space="PSUM")
                nc.tensor.matmul(out=tp[:], lhsT=gT[:, sl], rhs=ident_t[:],
                                 start=True, stop=True)
                gn = npool.tile([BLK, D], f16, tag="gn")
                nc.scalar.copy(out=gn[:], in_=tp[:])
                nc.sync.dma_start(out=cc_in[sl, :], in_=gn[:])

            # ---- hop 2: q = A g, fused epilogue ----
            def hop2_out(bb):
                sl = slice(bb * BLK, (bb + 1) * BLK)
                out_ps = psum.tile([128, BLK], f32, tag="outp", space="PSUM")
                nc.tensor.matmul(out=out_ps[:], lhsT=featT[:, sl], rhs=M_t[0][:],
                                 start=True, stop=False)
                nc.tensor.matmul(out=out_ps[:], lhsT=gT[:, sl], rhs=M_t[1][:],
                                 start=False, stop=False)
                nc.tensor.matmul(out=out_ps[:], lhsT=qS[:, sl], rhs=M_t[2][:],
                                 start=False, stop=True)
                ob = npool.tile([BLK, D], f32, tag="ob")
                nc.vector.tensor_tensor(out=ob[:], in0=out_ps[:], in1=bias_t[:],
                                        op=mybir.AluOpType.add)
                nc.sync.dma_start(out=out_d[sl, :], in_=ob[:])

            def hops_body():
                run_hop(featH_d, gT, hop1_out)
                if variant == "full":
                    nc.gpsimd.collective_compute(
                        "AllGather",
                        mybir.AluOpType.bypass,
                        ins=[cc_in.opt()],
                        outs=[cc_out.opt()],
                        replica_groups=[list(range(NCORE))],
                    )
                h2_src = featH_d if variant == "debug_nocc" else cc_out
                run_hop(h2_src, qS, hop2_out)

            if variant.startswith("timing") and reps > 1:
                with tc.For_i(0, reps, 1):
                    hops_body()
            else:
                hops_body()

    nc.compile()
    return nc


def kernel(feat, W, bias, lambda_max, src, dst):
    in_maps, plan = _prep(feat, W, bias, lambda_max, src, dst)
    nc = _build(plan)
    res = bass_utils.run_bass_kernel_spmd(nc, in_maps, core_ids=list(range(NCORE)))
    # stashed for external benchmarking harnesses (not used by the kernel)
    kernel.last_nc = nc
    kernel.last_in_maps = in_maps
    kernel.last_plan = plan
    out = np.concatenate([res.results[k]["out"] for k in range(NCORE)], axis=0)
    return np.ascontiguousarray(out[: plan["N"]]).astype(np.float32)


# revision 8
# speedup vs baseline: 3.8108x; 2.3262x over previous
"""ChebConv (K=3) Trainium2 kernel, 8-core SPMD — HBM-gather design.

Math: with lam = lambda_max, c1=-2/lam, c2=2/lam-1, d1=-4/lam, d2=4/lam-2 and
A = D^-1/2 A D^-1/2 (in-degree norm, clamped), the reference output is

    out = feat @ M0 + g @ M1 + q @ M2 + bias,   g = A feat, q = A g
    M0 = W0^T + c2 W1^T + (d2 c2 - 1) W2^T
    M1 = c1 W1^T + (d1 c2 + d2 c1) W2^T
    M2 = d1 c1 W2^T

Device strategy (one NEFF, SPMD on 8 cores):
  - dst nodes padded to a multiple of 8*128 and block-partitioned; 98 dst
    blocks per core. Edges bucketed by (dst block, src quarter) on host; each
    bucket padded to a multiple of 128 "edge tiles" (max over cores so the
    program is shared). NQUAR=4 quarters of 25088 src nodes respect the
    int16 gather index limit; bucket padding ~13%.
  - per hop, edge-source rows are gathered DIRECTLY FROM HBM (node-major
    feature table) with gpsimd.dma_gather(transpose=False), which emits
    EDGE-MAJOR tiles [128e x 128f] consumed by the scatter matmul with no
    further data movement. KEY perf facts (measured):
      * gather descriptor generation is Q7-core bound; SWDGE queue q is
        served by Q7 core pair (2q, 2q+1), so calls ROTATE across all 4
        queues => ~4x descriptor throughput (2.3 ns/idx vs 9 ns/idx).
      * transpose-mode gathers CORRUPT when run concurrently on different
        queues (shared XBAR spray state); non-transpose gathers are safe.
  - per edge tile: one fused DVE tensor_scalar builds the weighted one-hot
    (iota == dl) * w, w = norm[src]*norm[dst] (0 for padding); matmul
    lhsT=X_tile[e,f] rhs=onehot[e,d] accumulates the (dst block, quarter)
    partial in PSUM [f,d], which drains into a resident SBUF accumulator
    (gT for hop 1, qS for hop 2): first quarter via ACT copy, later
    quarters via DVE add.
  - hop-1 accumulators are transposed back to node-major via an identity
    matmul and written to a DRAM bounce buffer; one fp16 AllGather shares
    g across cores; hop 2 repeats the structure gathering from the
    AllGather output. Dense epilogue per block on TensorE with host-folded
    M0/M1/M2.
"""
import os
import sys

sys.path.insert(0, "/opt/trn_rl_repo")

import numpy as np

import concourse.bacc as bacc
import concourse.mybir as mybir
import concourse.tile as tile
from concourse import bass_utils

NCORE = 8
BLK = 128
D = 128
NQUAR = 4
NQUEUE = 4
CALL_TILES = 32                      # edge tiles per dma_gather call
CALL_IDX = CALL_TILES * BLK


def _prep(feat, W, bias, lambda_max, src, dst):
    """Host-side graph preprocessing. Returns per-core in_maps + plan."""
    N = feat.shape[0]
    E = src.shape[0]
    src = np.asarray(src).astype(np.int64)
    dst = np.asarray(dst).astype(np.int64)
    feat = np.asarray(feat).astype(np.float32)
    W = np.asarray(W).astype(np.float32)
    bias = np.asarray(bias).astype(np.float32)
    lam = float(np.asarray(lambda_max).reshape(-1)[0])

    npad_unit = NCORE * BLK
    NPAD = ((N + npad_unit - 1) // npad_unit) * npad_unit
    NBLK = NPAD // BLK
    BPC = NBLK // NCORE
    NPC = BPC * BLK
    QR = NPAD // NQUAR                # rows per quarter table
    assert QR - 1 < 32767, (NPAD, QR)

    # normalization
    deg = np.bincount(dst, minlength=N).astype(np.float32)
    norm = np.clip(deg, 1.0, None) ** -0.5
    w_all = (norm[src] * norm[dst]).astype(np.float32)

    blk_all = dst // BLK                      # global dst block
    quar_all = src // QR
    key = (blk_all * NQUAR + quar_all).astype(np.int64)
    order = np.argsort(key, kind="stable")
    sk = key[order]

    cnt_flat = np.bincount(key, minlength=NBLK * NQUAR)
    cnt = cnt_flat.reshape(NCORE, BPC, NQUAR)
    # tiles per (block-within-core, quarter): max over cores -> shared program
    T = -(-cnt.max(axis=0) // BLK)            # [BPC, NQUAR]
    # every block needs at least one tile so its accumulator gets written
    none_mask = T.sum(axis=1) == 0
    T[none_mask, 0] = 1
    tile_off = np.zeros((BPC, NQUAR), np.int64)
    NT = np.zeros(NQUAR, np.int64)
    for c in range(NQUAR):
        tile_off[:, c] = np.cumsum(T[:, c]) - T[:, c]
        NT[c] = T[:, c].sum()

    # slot position of every edge inside its core's per-quarter stream
    group_starts = np.zeros(NBLK * NQUAR + 1, np.int64)
    group_starts[1:] = np.cumsum(cnt_flat)
    rank = np.arange(E, dtype=np.int64) - group_starts[sk]
    bb_s = (sk // NQUAR) % BPC
    core_s = (sk // NQUAR) // BPC
    c_s = sk % NQUAR
    pos = tile_off[bb_s, c_s] * BLK + rank

    # quarter-local row index (the HBM gather idx)
    idx16_all = (src - quar_all * QR).astype(np.int16)[order]
    w_s = w_all[order]
    dl_s = (dst % BLK).astype(np.float32)[order]

    idxs = [np.zeros((NCORE, NT[c] * BLK), np.int16) for c in range(NQUAR)]
    ws = [np.zeros((NCORE, NT[c] * BLK), np.float32) for c in range(NQUAR)]
    dls = [np.zeros((NCORE, NT[c] * BLK), np.float32) for c in range(NQUAR)]
    for c in range(NQUAR):
        m = c_s == c
        idxs[c][core_s[m], pos[m]] = idx16_all[m]
        ws[c][core_s[m], pos[m]] = w_s[m]
        dls[c][core_s[m], pos[m]] = dl_s[m]

    # folded dense matrices
    c1 = -2.0 / lam
    c2 = 2.0 / lam - 1.0
    d1 = -4.0 / lam
    d2 = 4.0 / lam - 2.0
    W0T, W1T, W2T = W[0].T, W[1].T, W[2].T
    M0 = W0T + c2 * W1T + (d2 * c2 - 1.0) * W2T
    M1 = c1 * W1T + (d1 * c2 + d2 * c1) * W2T
    M2 = (d1 * c1) * W2T

    featH = np.zeros((NPAD, D), np.float16)
    featH[:N] = feat.astype(np.float16)

    # first/last nonempty quarter per block (shared across cores)
    first_c = np.zeros(BPC, np.int64)
    last_c = np.zeros(BPC, np.int64)
    for bb in range(BPC):
        nz = np.nonzero(T[bb])[0]
        first_c[bb] = nz[0]
        last_c[bb] = nz[-1]

    shared = {
        "M0": M0.astype(np.float16),
        "M1": M1.astype(np.float16),
        "M2": M2.astype(np.float16),
        "bias_rep": np.tile(bias[None, :].astype(np.float32), (BLK, 1)),
        "iota": np.tile(np.arange(BLK, dtype=np.float16)[None, :], (BLK, 1)),
        "ident": np.eye(BLK, dtype=np.float16),
        "featH": featH,
    }
    in_maps = []
    for k in range(NCORE):
        m = dict(shared)
        m["featLocal"] = featH[k * NPC : (k + 1) * NPC]
        for c in range(NQUAR):
            m[f"idx{c}"] = np.ascontiguousarray(
                np.tile(idxs[c][k].reshape(-1, 16).T, (8, 1))
            )
            m[f"w{c}"] = np.ascontiguousarray(ws[c][k].reshape(-1, BLK).T)
            m[f"dl{c}"] = np.ascontiguousarray(dls[c][k].reshape(-1, BLK).T)
        in_maps.append(m)

    plan = dict(N=N, NPAD=NPAD, BPC=BPC, NPC=NPC, QR=QR,
                T=T, tile_off=tile_off, NT=NT, first_c=first_c, last_c=last_c)
    return in_maps, plan


def _build(plan, variant="full", reps=1):
    """Emit the Bass/Tile program for the shared SPMD NEFF.

    variant="full": the real kernel (hop1 -> AllGather -> hop2+epilogue).
    variant="timing_*": no collective; hops wrapped in a For_i(reps)
    hardware loop for differential wall-clock timing.
    """
    BPC, NPC, NPAD, QR = plan["BPC"], plan["NPC"], plan["NPAD"], plan["QR"]
    T, tile_off, NT = plan["T"], plan["tile_off"], plan["NT"]
    first_c, last_c = plan["first_c"], plan["last_c"]
    f16, f32, i16 = mybir.dt.float16, mybir.dt.float32, mybir.dt.int16

    nc = bacc.Bacc("TRN2", target_bir_lowering=False, debug=False,
                   num_devices=NCORE, num_swdge_queues=NQUEUE)
    featH_d = nc.dram_tensor("featH", [NPAD, D], f16, kind="ExternalInput")
    featL_d = nc.dram_tensor("featLocal", [NPC, D], f16, kind="ExternalInput")
    idx_d = [nc.dram_tensor(f"idx{c}", [128, NT[c] * 8], i16, kind="ExternalInput")
             for c in range(NQUAR)]
    w_d = [nc.dram_tensor(f"w{c}", [128, NT[c]], f32, kind="ExternalInput")
           for c in range(NQUAR)]
    dl_d = [nc.dram_tensor(f"dl{c}", [128, NT[c]], f32, kind="ExternalInput")
            for c in range(NQUAR)]
    M_d = [nc.dram_tensor(f"M{i}", [D, D], f16, kind="ExternalInput")
           for i in range(3)]
    bias_d = nc.dram_tensor("bias_rep", [BLK, D], f32, kind="ExternalInput")
    iota_d = nc.dram_tensor("iota", [BLK, BLK], f16, kind="ExternalInput")
    ident_d = nc.dram_tensor("ident", [BLK, BLK], f16, kind="ExternalInput")
    out_d = nc.dram_tensor("out", [NPC, D], f32, kind="ExternalOutput")

    skip_gather = variant == "timing_compute"
    skip_compute = variant == "timing_gather"

    with tile.TileContext(nc) as tc:
        with (
            tc.tile_pool(name="const", bufs=1) as cpool,
            tc.tile_pool(name="resident", bufs=1) as rpool,
            tc.tile_pool(name="idxp", bufs=8) as idxpool,
            tc.tile_pool(name="streams", bufs=8) as spool,
            tc.tile_pool(name="ow", bufs=8) as owpool,
            tc.tile_pool(name="small", bufs=3) as npool,
            tc.tile_pool(name="psum", bufs=1, space="PSUM") as psum,
            tc.tile_pool(name="psum_iota", bufs=1, space="PSUM") as iotapsum,
            tc.tile_pool(name="psum_acc", bufs=4, space="PSUM") as acpsum,
            tc.tile_pool(name="dram", bufs=1, space="DRAM") as dram,
        ):
            iota_t = cpool.tile([BLK, BLK], f16)
            nc.sync.dma_start(out=iota_t[:], in_=iota_d[:])
            # f32 iota in PSUM: forces the one-hot tensor_scalar into 1x
            # memory-access mode (non-SBUF src), so DVE never grabs the
            # shared SBUF port pair and SWDGE descriptor gen is not starved.
            iota_ps = iotapsum.tile([BLK, BLK], f32, tag="iops", space="PSUM")
            nc.vector.tensor_copy(out=iota_ps[:], in_=iota_t[:])
            ident_t = cpool.tile([BLK, BLK], f16)
            nc.sync.dma_start(out=ident_t[:], in_=ident_d[:])
            M_t = []
            for i in range(3):
                t = cpool.tile([D, D], f16, tag=f"M{i}")
                nc.sync.dma_start(out=t[:], in_=M_d[i][:])
                M_t.append(t)
            bias_t = cpool.tile([BLK, D], f32)
            nc.sync.dma_start(out=bias_t[:], in_=bias_d[:])
            w_t, dl_t = [], []
            for c in range(NQUAR):
                wt = rpool.tile([128, NT[c]], f32, tag=f"w{c}")
                nc.sync.dma_start(out=wt[:], in_=w_d[c][:])
                w_t.append(wt)
                dt_ = rpool.tile([128, NT[c]], f32, tag=f"dl{c}")
                nc.sync.dma_start(out=dt_[:], in_=dl_d[c][:])
                dl_t.append(dt_)
            featT = rpool.tile([128, NPC], f16, tag="featT")
            nc.sync.dma_start_transpose(out=featT[:], in_=featL_d[:])
            gT = rpool.tile([128, NPC], f16, tag="gT")
            qS = rpool.tile([128, NPC], f16, tag="qS")

            cc_in = dram.tile([NPC, D], f16)
            cc_out = dram.tile([NPAD, D], f16)

            # position -> owning dst block, within each quarter's tile stream
            p2bb = []
            for c in range(NQUAR):
                arr = np.zeros(int(NT[c]), np.int64)
                for bb in range(BPC):
                    o = int(tile_off[bb][c])
                    arr[o : o + int(T[bb][c])] = bb
                p2bb.append(arr)

            def run_hop(src_table, accT, out_hook):
                """One SpMM hop: accT[:, bb*128:(bb+1)*128] = sum over
                quarters of the (bb, c) PSUM partials; out_hook(bb) emitted
                after the last quarter of bb drains."""
                qcounter = [0]
                for c in range(NQUAR):
                    ntc = int(NT[c])
                    ncalls = -(-ntc // CALL_TILES)
                    table_view = src_table[c * QR : (c + 1) * QR, :]
                    gbufs = {}

                    def ensure_call(j):
                        if j in gbufs or skip_gather:
                            return
                        n_t = min(CALL_TILES, ntc - j * CALL_TILES)
                        n_idx = n_t * BLK
                        ib = idxpool.tile([128, CALL_IDX // 16], i16, tag="idx")
                        nc.sync.dma_start(
                            out=ib[:, : n_idx // 16],
                            in_=idx_d[c][:, j * (CALL_IDX // 16):
                                         j * (CALL_IDX // 16) + n_idx // 16],
                        )
                        gb = spool.tile([128, CALL_TILES, D], f16, tag="g")
                        nc.gpsimd.dma_gather(
                            out_ap=gb[:, :n_t, :],
                            in_ap=table_view,
                            idxs_ap=ib[:, : n_idx // 16],
                            num_idxs=n_idx,
                            num_idxs_reg=n_idx,
                            elem_size=D,
                            transpose=False,
                            single_packet=False,
                            queue_num=qcounter[0] % NQUEUE,
                        )
                        qcounter[0] += 1
                        gbufs[j] = gb

                    if skip_compute:
                        for j in range(ncalls):
                            ensure_call(j)
                        continue

                    # static fake gather bufs for timing_compute
                    if skip_gather:
                        for b in range(2):
                            gb = spool.tile([128, CALL_TILES, D], f16, tag="g")
                            nc.vector.memset(gb[:], 0.0)
                            gbufs[b] = gb

                    for p in range(ntc):
                        j, slot = divmod(p, CALL_TILES)
                        if skip_gather:
                            gb = gbufs[j % 2]
                        else:
                            ensure_call(j)
                            gb = gbufs[j]
                        bb = int(p2bb[c][p])
                        start = p == int(tile_off[bb][c])
                        stop = p == int(tile_off[bb][c]) + int(T[bb][c]) - 1
                        if start:
                            acc = acpsum.tile([128, BLK], f32, tag="acc",
                                              space="PSUM")
                        ow = owpool.tile([128, BLK], f16, tag="ow")
                        nc.vector.tensor_scalar(
                            out=ow[:],
                            in0=iota_ps[:],
                            scalar1=dl_t[c][:, p : p + 1],
                            scalar2=w_t[c][:, p : p + 1],
                            op0=mybir.AluOpType.is_equal,
                            op1=mybir.AluOpType.mult,
                        )
                        nc.tensor.matmul(
                            out=acc[:],
                            lhsT=gb[:, slot, :],
                            rhs=ow[:],
                            start=start,
                            stop=stop,
                        )
                        if stop:
                            sl = slice(bb * BLK, (bb + 1) * BLK)
                            if c == int(first_c[bb]):
                                nc.scalar.copy(out=accT[:, sl], in_=acc[:])
                            else:
                                nc.vector.tensor_tensor(
                                    out=accT[:, sl], in0=accT[:, sl], in1=acc[:],
                                    op=mybir.AluOpType.add)
                            if c == int(last_c[bb]):
                                out_hook(bb)

            # ---- hop 1: g = A feat ----
            def hop1_out(bb):
                sl = slice(bb * BLK, (bb + 1) * BLK)
                tp = psum.tile([128, BLK], f32, tag="tp", space="PSUM")
                nc.tensor.matmul(out=tp[:], lhsT=gT[:, sl], rhs=ident_t[:],
                                 start=True, stop=True)
                gn = npool.tile([BLK, D], f16, tag="gn")
                nc.scalar.copy(out=gn[:], in_=tp[:])
                nc.sync.dma_start(out=cc_in[sl, :], in_=gn[:])

            # ---- hop 2: q = A g, fused epilogue ----
            def hop2_out(bb):
                sl = slice(bb * BLK, (bb + 1) * BLK)
                out_ps = psum.tile([128, BLK], f32, tag="outp", space="PSUM")
                nc.tensor.matmul(out=out_ps[:], lhsT=featT[:, sl], rhs=M_t[0][:],
                                 start=True, stop=False)
                nc.tensor.matmul(out=out_ps[:], lhsT=gT[:, sl], rhs=M_t[1][:],
                                 start=False, stop=False)
                nc.tensor.matmul(out=out_ps[:], lhsT=qS[:, sl], rhs=M_t[2][:],
                                 start=False, stop=True)
                ob = npool.tile([BLK, D], f32, tag="ob")
                nc.vector.tensor_tensor(out=ob[:], in0=out_ps[:], in1=bias_t[:],
                                        op=mybir.AluOpType.add)
                nc.sync.dma_start(out=out_d[sl, :], in_=ob[:])

            def hops_body():
                run_hop(featH_d, gT, hop1_out)
                if variant == "full":
                    nc.gpsimd.collective_compute(
                        "AllGather",
                        mybir.AluOpType.bypass,
                        ins=[cc_in.opt()],
                        outs=[cc_out.opt()],
                        replica_groups=[list(range(NCORE))],
                    )
                h2_src = featH_d if variant == "debug_nocc" else cc_out
                run_hop(h2_src, qS, hop2_out)

            if variant.startswith("timing") and reps > 1:
                with tc.For_i(0, reps, 1):
                    hops_body()
            else:
                hops_body()

    nc.compile()
    return nc


def kernel(feat, W, bias, lambda_max, src, dst):
    in_maps, plan = _prep(feat, W, bias, lambda_max, src, dst)
    nc = _build(plan)
    res = bass_utils.run_bass_kernel_spmd(nc, in_maps, core_ids=list(range(NCORE)))
    # stashed for external benchmarking harnesses (not used by the kernel)
    kernel.last_nc = nc
    kernel.last_in_maps = in_maps
    kernel.last_plan = plan
    out = np.concatenate([res.results[k]["out"] for k in range(NCORE)], axis=0)
    return np.ascontiguousarray(out[: plan["N"]]).astype(np.float32)
